# revision 3
# baseline (speedup 1.0000x reference)
"""Depthwise causal conv1d (W=8) with 3 interleaved weight sets, on 8 TRN2 cores.

Reference computes r/o/a = depthwise_causal_conv(x, {rtg,obs,act}_{w,b}) and
interleaves out[:, t] = {r,o,a}[:, t] by t % 3.  Only the t%3-matching third of
each conv is needed, so total work is exactly one conv: for each output t,
out[b,t,h] = sum_k x[b, t-7+k, h] * w_{t%3}[h, k] + b_{t%3}[h].

Strategy (pure batch data-parallel, B=16 -> 2 per core):
  - host pre-transposes x to channels-major [2, 6, 128, T] per core
  - on-chip: per (batch, channel-group) tile [128c, T+7pad] in SBUF; the conv
    runs on the TensorEngine as 8 accumulating fp32r matmuls per output block:
    lhsT = diag(w_set[:, k]) (128x128), rhs = the x tile itself at free-dim
    offset s+k with stride 3 (the t%3 decimation), accumulated in PSUM
  - ScalarE evicts PSUM with fused per-channel bias and strided (stride-3)
    writes that re-interleave the 3 phases into the final [128, T] tile
  - host transposes back.
"""

import os
import numpy as np

B, T, H, W = 16, 3072, 768, 8
NCORES = 8
B_LOC = B // NCORES          # 2 batches per core
G = H // 128                 # 6 channel groups
TP = T + W - 1               # left-padded time
NFREE = 512                  # psum tile width (one bank of fp32)
NT = (T // 3) // NFREE       # 2 psum tiles per phase

_cache = {}


def _build_nc():
    import concourse.bacc as bacc
    import concourse.mybir as mybir
    import concourse.tile as tile

    nc = bacc.Bacc("TRN2", target_bir_lowering=False, debug=False)
    f32 = mybir.dt.float32
    f32r = mybir.dt.float32r

    x_d = nc.dram_tensor("x", [B_LOC, G, 128, T], f32r, kind="ExternalInput").ap()
    wid_d = nc.dram_tensor("wid", [128, 128], f32, kind="ExternalInput").ap()
    w_d = nc.dram_tensor("w", [128, G * 3 * W], f32, kind="ExternalInput").ap()
    b_d = nc.dram_tensor("b", [128, G * 3], f32, kind="ExternalInput").ap()
    y_d = nc.dram_tensor("y", [B_LOC, G, 128, T], f32, kind="ExternalOutput").ap()

    with tile.TileContext(nc) as tc:
        with (
            tc.tile_pool(name="const", bufs=1) as constp,
            tc.tile_pool(name="diag", bufs=2) as diagp,
            tc.tile_pool(name="xp", bufs=3) as xp,
            tc.tile_pool(name="op", bufs=3) as op,
            tc.tile_pool(name="ps", bufs=6, space="PSUM") as psp,
        ):
            wid = constp.tile([128, 128], f32)
            wt = constp.tile([128, G * 3 * W], f32)
            bt = constp.tile([128, G * 3], f32)
            nc.sync.dma_start(wid[:], wid_d[:])
            nc.sync.dma_start(wt[:], w_d[:])
            nc.sync.dma_start(bt[:], b_d[:])

            for g in range(G):
                # the 24 diagonal weight matrices for this channel group
                diags = []
                for s in range(3):
                    for k in range(W):
                        dt_ = diagp.tile([128, 128], f32r, tag=f"diag{s}_{k}")
                        nc.vector.tensor_scalar_mul(
                            dt_[:], wid[:], wt[:, (g * 3 + s) * W + k : (g * 3 + s) * W + k + 1]
                        )
                        diags.append(dt_)
                for b in range(B_LOC):
                    xt = xp.tile([128, TP], f32r)
                    nc.vector.memset(xt[:, 0 : W - 1].bitcast(mybir.dt.uint32), 0)
                    nc.sync.dma_start(xt[:, W - 1 : TP], x_d[b, g])
                    ot = op.tile([128, T], f32)
                    for s in range(3):
                        for nt in range(NT):
                            ps = psp.tile([128, NFREE], f32)
                            for k in range(W):
                                off = s + k + 3 * NFREE * nt
                                rhs = xt[:, off : off + 3 * (NFREE - 1) + 1 : 3]
                                nc.tensor.matmul(
                                    ps[:], diags[s * W + k][:], rhs,
                                    start=(k == 0), stop=(k == W - 1),
                                )
                            d0 = s + 3 * NFREE * nt
                            dst = ot[:, d0 : d0 + 3 * (NFREE - 1) + 1 : 3]
                            nc.scalar.activation(
                                dst, ps[:], mybir.ActivationFunctionType.Identity,
                                bias=bt[:, g * 3 + s : g * 3 + s + 1], scale=1.0,
                            )
                    nc.sync.dma_start(y_d[b, g], ot[:])

    nc.compile()
    return nc


def _get_nc():
    if "nc" not in _cache:
        _cache["nc"] = _build_nc()
    return _cache["nc"]


def _install_ntff_hook():
    """antenv.axon_hooks is not shipped in this container; shim it so
    bass_utils can find the NTFF profile hook (trace=True path)."""
    import sys, types
    if "antenv.axon_hooks" in sys.modules:
        return
    mod = types.ModuleType("antenv.axon_hooks")
    mod._hook = None
    mod.set_axon_ntff_profile_hook = lambda h: setattr(mod, "_hook", h)
    mod.get_axon_ntff_profile_hook = lambda: mod._hook
    sys.modules["antenv.axon_hooks"] = mod
    try:
        from trn_agent_boot.trn_boot import _ntff_profile_via_ctypes
        mod._hook = _ntff_profile_via_ctypes("/opt/axon/libaxon_pjrt.so")
    except Exception:
        mod._hook = None


def kernel(x, rtg_w, rtg_b, obs_w, obs_b, act_w, act_b):
    from concourse import bass_utils

    x = np.asarray(x, dtype=np.float32)
    w_sets = [np.asarray(a, dtype=np.float32) for a in (rtg_w, obs_w, act_w)]
    b_sets = [np.asarray(a, dtype=np.float32) for a in (rtg_b, obs_b, act_b)]

    # weights laid out [128 c_local, (g*3+s)*8+k]; biases [128, g*3+s]
    w_all = np.zeros((128, G * 3 * W), dtype=np.float32)
    b_all = np.zeros((128, G * 3), dtype=np.float32)
    for g in range(G):
        for s in range(3):
            w_all[:, (g * 3 + s) * W : (g * 3 + s + 1) * W] = w_sets[s][g * 128 : (g + 1) * 128]
            b_all[:, g * 3 + s] = b_sets[s][g * 128 : (g + 1) * 128]
    wid = np.eye(128, dtype=np.float32)

    in_maps = []
    for c in range(NCORES):
        xc = x[c * B_LOC : (c + 1) * B_LOC]                      # [2, T, H]
        x_t = np.ascontiguousarray(xc.transpose(0, 2, 1))        # [2, H, T]
        x_t = x_t.reshape(B_LOC, G, 128, T)
        in_maps.append({"x": x_t, "wid": wid, "w": w_all, "b": b_all})

    nc = _get_nc()
    trace = bool(int(os.environ.get("KERNEL_TRACE", "0")))
    if trace:
        _install_ntff_hook()
    res = bass_utils.run_bass_kernel_spmd(
        nc, in_maps, core_ids=list(range(NCORES)), trace=trace,
    )
    _cache["last_result"] = res

    out = np.empty((B, T, H), dtype=np.float32)
    for c in range(NCORES):
        y_t = res.results[c]["y"].reshape(B_LOC, H, T)           # [2, H, T]
        out[c * B_LOC : (c + 1) * B_LOC] = y_t.transpose(0, 2, 1)
    return out


# revision 4
# speedup vs baseline: 1.2924x; 1.2924x over previous
"""Depthwise causal conv1d (W=8) with 3 interleaved weight sets, on 8 TRN2 cores.

Reference computes r/o/a = depthwise_causal_conv(x, {rtg,obs,act}_{w,b}) and
interleaves out[:, t] = {r,o,a}[:, t] by t % 3.  Only the t%3-matching third of
each conv is needed, so total work is exactly one conv: for each output t,
out[b,t,h] = sum_k x[b, t-7+k, h] * w_{t%3}[h, k] + b_{t%3}[h].

Strategy (pure batch data-parallel, B=16 -> 2 per core):
  - host pre-transposes x to channels-major [2, 6, 128, T] per core, cast fp16
  - on-chip: per (batch, channel-group) tile [128c, T+7pad] in SBUF; the conv
    runs on the TensorEngine as 8 accumulating fp16 matmuls per output block:
    lhsT = diag(w_set[:, k]) (128x128), rhs = the x tile itself at free-dim
    offset s+k with stride 3 (the t%3 decimation), accumulated fp32 in PSUM
  - ScalarE evicts PSUM with fused per-channel f32 bias and strided (stride-3)
    fp16 writes that re-interleave the 3 phases into the final [128, T] tile
  - host transposes back / upcasts to f32.
fp16 end-to-end rel err ~5e-4 (x, w quantization + fp16 output rounding).
"""

import os
import numpy as np

B, T, H, W = 16, 3072, 768, 8
NCORES = 8
B_LOC = B // NCORES          # 2 batches per core
G = H // 128                 # 6 channel groups
TP = T + W - 1               # left-padded time
NFREE = 512                  # psum tile width (one fp32 bank)
NT = (T // 3) // NFREE       # 2 psum tiles per phase

_cache = {}


def _build_nc():
    import concourse.bacc as bacc
    import concourse.mybir as mybir
    import concourse.tile as tile

    nc = bacc.Bacc("TRN2", target_bir_lowering=False, debug=False)
    f32 = mybir.dt.float32
    f16 = mybir.dt.float16

    x_d = nc.dram_tensor("x", [B_LOC, G, 128, T], f16, kind="ExternalInput").ap()
    wid_d = nc.dram_tensor("wid", [128, 128], f16, kind="ExternalInput").ap()
    w_d = nc.dram_tensor("w", [128, G * 3 * W], f32, kind="ExternalInput").ap()
    b_d = nc.dram_tensor("b", [128, G * 3], f32, kind="ExternalInput").ap()
    y_d = nc.dram_tensor("y", [B_LOC, G, 128, T], f16, kind="ExternalOutput").ap()

    with tile.TileContext(nc) as tc:
        with (
            tc.tile_pool(name="const", bufs=1) as constp,
            tc.tile_pool(name="diag", bufs=2) as diagp,
            tc.tile_pool(name="xp", bufs=3) as xp,
            tc.tile_pool(name="op", bufs=3) as op,
            tc.tile_pool(name="ps", bufs=6, space="PSUM") as psp,
        ):
            wid = constp.tile([128, 128], f16)
            wt = constp.tile([128, G * 3 * W], f32)
            bt = constp.tile([128, G * 3], f32)
            nc.sync.dma_start(wid[:], wid_d[:])
            nc.sync.dma_start(wt[:], w_d[:])
            nc.sync.dma_start(bt[:], b_d[:])

            for g in range(G):
                # the 24 diagonal fp16 weight matrices for this channel group
                diags = []
                for s in range(3):
                    for k in range(W):
                        c = (g * 3 + s) * W + k
                        dt_ = diagp.tile([128, 128], f16, tag=f"diag{s}_{k}")
                        nc.vector.tensor_scalar_mul(dt_[:], wid[:], wt[:, c : c + 1])
                        diags.append(dt_)
                for b in range(B_LOC):
                    xt = xp.tile([128, TP], f16)
                    nc.vector.memset(xt[:, 0 : W - 1], 0)
                    nc.sync.dma_start(xt[:, W - 1 : TP], x_d[b, g])
                    ot = op.tile([128, T], f16)
                    for s in range(3):
                        for nt in range(NT):
                            ps = psp.tile([128, NFREE], f32)
                            for k in range(W):
                                off = s + k + 3 * NFREE * nt
                                rhs = xt[:, off : off + 3 * (NFREE - 1) + 1 : 3]
                                nc.tensor.matmul(
                                    ps[:], diags[s * W + k][:], rhs,
                                    start=(k == 0), stop=(k == W - 1),
                                )
                            d0 = s + 3 * NFREE * nt
                            dst = ot[:, d0 : d0 + 3 * (NFREE - 1) + 1 : 3]
                            nc.scalar.activation(
                                dst, ps[:], mybir.ActivationFunctionType.Identity,
                                bias=bt[:, g * 3 + s : g * 3 + s + 1], scale=1.0,
                            )
                    nc.sync.dma_start(y_d[b, g], ot[:])

    nc.compile()
    return nc


def _get_nc():
    if "nc" not in _cache:
        _cache["nc"] = _build_nc()
    return _cache["nc"]


def _install_ntff_hook():
    """antenv.axon_hooks is not shipped in this container; shim it so
    bass_utils can find the NTFF profile hook (trace=True path)."""
    import sys, types
    if "antenv.axon_hooks" in sys.modules:
        return
    mod = types.ModuleType("antenv.axon_hooks")
    mod._hook = None
    mod.set_axon_ntff_profile_hook = lambda h: setattr(mod, "_hook", h)
    mod.get_axon_ntff_profile_hook = lambda: mod._hook
    sys.modules["antenv.axon_hooks"] = mod
    try:
        from trn_agent_boot.trn_boot import _ntff_profile_via_ctypes
        mod._hook = _ntff_profile_via_ctypes("/opt/axon/libaxon_pjrt.so")
    except Exception:
        mod._hook = None


def kernel(x, rtg_w, rtg_b, obs_w, obs_b, act_w, act_b):
    from concourse import bass_utils

    x = np.asarray(x, dtype=np.float32)
    w_sets = [np.asarray(a, dtype=np.float32) for a in (rtg_w, obs_w, act_w)]
    b_sets = [np.asarray(a, dtype=np.float32) for a in (rtg_b, obs_b, act_b)]

    # fp16-rounded weights, laid out [128 c_local, (g*3+s)*8+k] as f32 values
    # (the on-chip diag build multiplies an fp16 identity by this f32 scalar)
    w_all = np.zeros((128, G * 3 * W), dtype=np.float32)
    b_all = np.zeros((128, G * 3), dtype=np.float32)
    for g in range(G):
        for s in range(3):
            w_all[:, (g * 3 + s) * W : (g * 3 + s + 1) * W] = w_sets[s][g * 128 : (g + 1) * 128]
            b_all[:, g * 3 + s] = b_sets[s][g * 128 : (g + 1) * 128]
    wid = np.eye(128, dtype=np.float16)

    in_maps = []
    for c in range(NCORES):
        xc = x[c * B_LOC : (c + 1) * B_LOC]                      # [2, T, H]
        x_t = np.ascontiguousarray(xc.transpose(0, 2, 1))        # [2, H, T]
        x_t = x_t.reshape(B_LOC, G, 128, T).astype(np.float16)
        in_maps.append({"x": x_t, "wid": wid, "w": w_all, "b": b_all})

    nc = _get_nc()
    trace = bool(int(os.environ.get("KERNEL_TRACE", "0")))
    if trace:
        _install_ntff_hook()
    res = bass_utils.run_bass_kernel_spmd(
        nc, in_maps, core_ids=list(range(NCORES)), trace=trace,
    )
    _cache["last_result"] = res

    out = np.empty((B, T, H), dtype=np.float32)
    for c in range(NCORES):
        y_t = res.results[c]["y"].astype(np.float32).reshape(B_LOC, H, T)
        out[c * B_LOC : (c + 1) * B_LOC] = y_t.transpose(0, 2, 1)
    return out
